# revision 47
# baseline (speedup 1.0000x reference)
"""Causal multi-head attention (B=4, S=2048, D=768, H=12, Dh=64) on 8 TRN2 NeuronCores.

Sharding: B x head-group. Core c handles batch b = c//2, heads 6g..6g+5 with
g = c%2. Each core computes QKV projections for its 6 heads, causal
flash-style attention in scores-transposed layout, and a partial W_O
contraction. Host sums the two per-batch partials and adds b_O.

Structure: j-major over q-blocks, processing heads in PAIRS (even head in
partitions 0-63, odd head in 64-127) so the two score matmuls (contraction
Dh=64) run concurrently in different PE row-groups. Projection / W_O matmuls
interleave as fillers to keep TensorE dense. Warmup matmuls at t=0 warm the
HAM clock gate while input DMAs (split across both queues, d-chunk-major)
are in flight.

No collectives: per-core outputs are disjoint-summable partials.
"""
import sys

if "/opt/trn_rl_repo" not in sys.path:
    sys.path.insert(0, "/opt/trn_rl_repo")

import contextlib

import ml_dtypes
import numpy as np

import concourse.bass as bass
import concourse.tile as tile
from concourse import bacc, mybir
from concourse import bass_utils

F32 = mybir.dt.float32
F32R = mybir.dt.float32r
BF16 = mybir.dt.bfloat16
FP8 = mybir.dt.float8e4
DR = mybir.MatmulPerfMode.DoubleRow
Exp = mybir.ActivationFunctionType.Exp

B, S, D, H, Dh = 4, 2048, 768, 12, 64
HL = 6          # heads per core
NE = HL * Dh    # 384 he-dims per core
NC_D = D // 128   # 6 d chunks
NC_E = NE // 128  # 3 he chunks
QB = 512        # q block
NQB = S // QB   # 4
NKT = S // 128  # 16 k tiles
VW = Dh + 1     # 65: v + ones column
SCALE = 1.0 / np.sqrt(Dh)
W8SC = 32.0     # fp8 weight prescale; q,k come out 32x large
SCALE8 = SCALE / (W8SC * W8SC)  # folds the 32^2 back out inside the exp
NWARM = 10      # HAM warmup matmuls

_CACHE = {}


def _build():
    nc = bacc.Bacc("TRN2", target_bir_lowering=False, debug=False, num_devices=8)
    xt_d = nc.dram_tensor("xt", [D, S], BF16, kind="ExternalInput")
    xt8_d = nc.dram_tensor("xt8", [D, S], FP8, kind="ExternalInput")
    wq_d = nc.dram_tensor("wq", [D, NE], FP8, kind="ExternalInput")
    wk_d = nc.dram_tensor("wk", [D, NE], FP8, kind="ExternalInput")
    wv_d = nc.dram_tensor("wv", [D, NE], BF16, kind="ExternalInput")
    wo_d = nc.dram_tensor("wo", [NE, D], BF16, kind="ExternalInput")
    bq_d = nc.dram_tensor("bq", [128, NC_E], F32, kind="ExternalInput")
    bk_d = nc.dram_tensor("bk", [128, NC_E], F32, kind="ExternalInput")
    bv_d = nc.dram_tensor("bv", [128, HL * VW], F32, kind="ExternalInput")
    mask_d = nc.dram_tensor("mask", [128, 128], BF16, kind="ExternalInput")
    out_d = nc.dram_tensor("out", [S, D], F32, kind="ExternalOutput")

    with tile.TileContext(nc) as tc:
        with contextlib.ExitStack() as ctx:
            sb = ctx.enter_context(tc.tile_pool(name="sb", bufs=1))
            pt_pool = ctx.enter_context(tc.tile_pool(name="pt", bufs=8))
            sm_pool = ctx.enter_context(tc.tile_pool(name="sm", bufs=6))
            o_pool = ctx.enter_context(tc.tile_pool(name="o", bufs=4))
            ps_s = ctx.enter_context(tc.tile_pool(name="pss", bufs=2, space="PSUM"))
            ps_mm = ctx.enter_context(tc.tile_pool(name="psmm", bufs=2, space="PSUM"))
            ps_z = ctx.enter_context(tc.tile_pool(name="psz", bufs=2, space="PSUM"))

            # ---- persistent SBUF ----
            xt = sb.tile([128, NC_D * S], BF16, tag="xt")
            xt8 = sb.tile([128, NC_D * S], FP8, tag="xt8")
            wq = sb.tile([128, NC_D * NE], FP8, tag="wq")
            wk = sb.tile([128, NC_D * NE], FP8, tag="wk")
            wv = sb.tile([128, NC_D * NE], BF16, tag="wv")
            wo = sb.tile([128, NC_E * D], BF16, tag="wo")
            bq = sb.tile([128, NC_E], F32, tag="bq")
            bk = sb.tile([128, NC_E], F32, tag="bk")
            bvb = sb.tile([128, HL * VW], F32, tag="bvb")
            mask = sb.tile([128, 128], BF16, tag="mask")
            ones_b = sb.tile([1, 64], BF16, tag="ones_b")
            wscr = sb.tile([128, 512], BF16, tag="wscr")
            qt = sb.tile([128, NC_E * S], BF16, tag="qt")
            kt = sb.tile([128, NC_E * S], BF16, tag="kt")
            va = sb.tile([128, NKT * HL * VW], BF16, tag="va")
            znt = sb.tile([128, NC_E * S], BF16, tag="znt")

            # ---- input DMAs: d-chunk-major groups split across both queues ----
            wq_r = wq_d.ap().rearrange("(c p) e -> p c e", p=128)
            wk_r = wk_d.ap().rearrange("(c p) e -> p c e", p=128)
            wv_r = wv_d.ap().rearrange("(c p) e -> p c e", p=128)
            wq_s = wq[:].rearrange("p (c e) -> p c e", c=NC_D)
            wk_s = wk[:].rearrange("p (c e) -> p c e", c=NC_D)
            wv_s = wv[:].rearrange("p (c e) -> p c e", c=NC_D)
            xt_r = xt_d.ap().rearrange("(c p) s -> p c s", p=128)
            xt_s = xt[:].rearrange("p (c s) -> p c s", c=NC_D)
            xt8_r = xt8_d.ap().rearrange("(c p) s -> p c s", p=128)
            xt8_s = xt8[:].rearrange("p (c s) -> p c s", c=NC_D)

            def load_group(q, c):
                q.dma_start(wq_s[:, c:c + 1, :], wq_r[:, c:c + 1, :])
                q.dma_start(wk_s[:, c:c + 1, :], wk_r[:, c:c + 1, :])
                q.dma_start(wv_s[:, c:c + 1, :], wv_r[:, c:c + 1, :])
                q.dma_start(xt_s[:, c, 0:QB], xt_r[:, c, 0:QB])
                q.dma_start(xt8_s[:, c, 0:QB], xt8_r[:, c, 0:QB])

            # alternate d-chunk groups across the two queues so each DoubleRow
            # chunk-PAIR (2k, 2k+1) completes as early as possible
            load_group(nc.sync, 0)
            load_group(nc.gpsimd, 1)
            load_group(nc.sync, 2)
            load_group(nc.gpsimd, 3)
            nc.sync.dma_start(bq[:], bq_d.ap())
            nc.sync.dma_start(bk[:], bk_d.ap())
            load_group(nc.sync, 4)
            load_group(nc.gpsimd, 5)
            nc.sync.dma_start(mask[:], mask_d.ap())
            nc.gpsimd.dma_start(bvb[:], bv_d.ap())
            wo_r = wo_d.ap().rearrange("(c p) d -> p c d", p=128)
            wo_s = wo[:].rearrange("p (c d) -> p c d", c=NC_E)
            # remaining x quarters (needed from j0 fillers on), then W_O weights
            # (not needed until late j2)
            for qq in range(1, 4):
                s0 = qq * QB
                nc.sync.dma_start(xt_s[:, 0:3, s0:s0 + QB], xt_r[:, 0:3, s0:s0 + QB])
                nc.gpsimd.dma_start(xt_s[:, 3:6, s0:s0 + QB], xt_r[:, 3:6, s0:s0 + QB])
                nc.sync.dma_start(xt8_s[:, 0:3, s0:s0 + QB], xt8_r[:, 0:3, s0:s0 + QB])
                nc.gpsimd.dma_start(xt8_s[:, 3:6, s0:s0 + QB], xt8_r[:, 3:6, s0:s0 + QB])
            nc.gpsimd.dma_start(wo_s[:], wo_r[:])

            # ---- constants + HAM warmup (no DMA deps) ----
            nc.vector.memset(ones_b[:], 1.0)
            nc.vector.memset(wscr[:], 0.0)
            va_4d = va[:].rearrange("p (s h e) -> p s h e", s=NKT, h=HL)
            nc.vector.memset(va_4d[:, :, :, Dh:Dh + 1], 1.0)
            wps = ps_z.tile([128, QB], F32, tag="z", name="wps")
            for _ in range(NWARM):
                nc.tensor.matmul(wps[:], wscr[:, 0:128], wscr[:], start=True,
                                 stop=True, skip_group_check=True)

            # ---- prologue: QK+V projections for q-block 0 / k-tiles 0-3 ----
            # 3 QK chains (ce 0-2) + V chains, all kc-progressive so they
            # consume the d-chunk DMA groups as they land.
            psq0 = ps_mm.tile([128, QB], F32, tag="mm", name="psq0")
            psk0 = ps_mm.tile([128, QB], F32, tag="mm", name="psk0")
            qk1 = ps_s.tile([128, 2 * QB], F32, tag="s", name="qk1")
            qk2 = ps_s.tile([128, 2 * QB], F32, tag="s", name="qk2")
            pro_qk = {0: (psq0[:], psk0[:]),
                      1: (qk1[0:128, 0:QB], qk1[0:128, QB:2 * QB]),
                      2: (qk2[0:128, 0:QB], qk2[0:128, QB:2 * QB])}
            psvs = [ps_z.tile([128, NE], F32, tag="z", name=f"psv{st}")
                    for st in range(4)]

            def v_bias_epilogue(st, psv):
                o = st * HL * VW
                va_v = va[:, o:o + HL * VW].rearrange("p (h e) -> p h e", h=HL)[:, :, 0:Dh]
                ps_v = psv.rearrange("p (h e) -> p h e", h=HL)
                bv_v = bvb[:].rearrange("p (h e) -> p h e", h=HL)[:, :, 0:Dh]
                nc.vector.tensor_add(va_v, ps_v, bv_v)

            for kc in range(NC_D):
                st_, sp = (kc == 0), (kc == NC_D - 1)
                if kc % 2 == 1:
                    # fp8 DoubleRow: d-chunk pair (kc-1, kc) contracts in one MM
                    kcp = kc // 2
                    for ce in range(NC_E):
                        pq, pk = pro_qk[ce]
                        lq = wq_s[:, 2 * kcp:2 * kcp + 2, ce * 128:ce * 128 + 128]
                        lk = wk_s[:, 2 * kcp:2 * kcp + 2, ce * 128:ce * 128 + 128]
                        r8 = xt8_s[:, 2 * kcp:2 * kcp + 2, 0:QB]
                        nc.tensor.matmul(pq, lq, r8, start=(kcp == 0),
                                         stop=(kcp == NC_D // 2 - 1),
                                         perf_mode=DR, skip_group_check=True)
                        nc.tensor.matmul(pk, lk, r8, start=(kcp == 0),
                                         stop=(kcp == NC_D // 2 - 1),
                                         perf_mode=DR, skip_group_check=True)
                        if sp:
                            nc.vector.tensor_scalar_add(qt[:, ce * S:ce * S + QB],
                                                        pq, bq[:, ce:ce + 1])
                            nc.vector.tensor_scalar_add(kt[:, ce * S:ce * S + QB],
                                                        pk, bk[:, ce:ce + 1])
                for st in range(4):
                    lx = xt[:, kc * S + st * 128:kc * S + st * 128 + 128]
                    nc.tensor.matmul(psvs[st][:], lx, wv[:, kc * NE:kc * NE + NE],
                                     start=st_, stop=sp)
                    if sp:
                        v_bias_epilogue(st, psvs[st][:])

            # ---- filler work units (each emits one PE matmul + epilogue) ----
            def qk_proj_units(sblk):
                s0 = sblk * QB
                for ce in range(NC_E):
                    state = {}

                    def unit(ce=ce, state=state):
                        kcp = state.setdefault("kcp", 0)
                        if kcp == 0:
                            state["psq"] = ps_mm.tile([128, QB], F32, tag="mm", name="psq")
                            state["psk"] = ps_mm.tile([128, QB], F32, tag="mm", name="psk")
                        lq = wq_s[:, 2 * kcp:2 * kcp + 2, ce * 128:ce * 128 + 128]
                        lk = wk_s[:, 2 * kcp:2 * kcp + 2, ce * 128:ce * 128 + 128]
                        r8 = xt8_s[:, 2 * kcp:2 * kcp + 2, s0:s0 + QB]
                        st_, sp = (kcp == 0), (kcp == NC_D // 2 - 1)
                        nc.tensor.matmul(state["psq"][:], lq, r8, start=st_, stop=sp,
                                         perf_mode=DR)
                        nc.tensor.matmul(state["psk"][:], lk, r8, start=st_, stop=sp,
                                         perf_mode=DR)
                        if sp:
                            nc.vector.tensor_scalar_add(
                                qt[:, ce * S + s0:ce * S + s0 + QB], state["psq"][:],
                                bq[:, ce:ce + 1])
                            nc.vector.tensor_scalar_add(
                                kt[:, ce * S + s0:ce * S + s0 + QB], state["psk"][:],
                                bk[:, ce:ce + 1])
                        state["kcp"] = kcp + 1

                    for _ in range(NC_D // 2):
                        yield unit

            def v_proj_units(st):
                state = {}

                def unit(state=state):
                    kc = state.setdefault("kc", 0)
                    if kc == 0:
                        state["psv"] = ps_mm.tile([128, NE], F32, tag="mm", name="psv")
                    lx = xt[:, kc * S + st * 128:kc * S + st * 128 + 128]
                    nc.tensor.matmul(state["psv"][:], lx, wv[:, kc * NE:kc * NE + NE],
                                     start=(kc == 0), stop=(kc == NC_D - 1))
                    if kc == NC_D - 1:
                        v_bias_epilogue(st, state["psv"][:])
                    state["kc"] = kc + 1

                for _ in range(NC_D):
                    yield unit

            def wo_units(st):
                osb = [None]

                def mkunit(dh, c, osb=osb):
                    def unit():
                        if dh == 0 and c == 0:
                            osb[0] = o_pool.tile([128, D], F32, tag="osb", name="osb")
                        if c == 0:
                            osb.append(ps_mm.tile([128, 384], F32, tag="mm", name="pso"))
                        pso = osb[-1]
                        lhsT = znt[:, c * S + st * 128:c * S + st * 128 + 128]
                        rhs = wo[:, c * D + dh * 384:c * D + dh * 384 + 384]
                        nc.tensor.matmul(pso[:], lhsT, rhs, start=(c == 0),
                                         stop=(c == NC_E - 1))
                        if c == NC_E - 1:
                            nc.vector.tensor_copy(osb[0][:, dh * 384:dh * 384 + 384], pso[:])
                            nc.sync.dma_start(
                                out_d.ap()[st * 128:st * 128 + 128,
                                           dh * 384:dh * 384 + 384],
                                osb[0][:, dh * 384:dh * 384 + 384])
                    return unit

                for dh in range(2):
                    for c in range(NC_E):
                        yield mkunit(dh, c)

            def rr(*gens):
                gens = [iter(g) for g in gens]
                out = []
                while gens:
                    nxt = []
                    for g in gens:
                        try:
                            out.append(next(g))
                            nxt.append(g)
                        except StopIteration:
                            pass
                    gens = nxt
                return out

            # ---- epilogue: normalize z^T by the softmax denominators ----
            def epilogue(h, j, zps, sc_copy=False):
                ce, sub = h // 2, h % 2
                p0 = 64 * sub
                q0 = j * QB
                lsb = sm_pool.tile([1, QB], F32, tag="lsb")
                if sc_copy:
                    # ScalarE is idle at the very end; lets the two final
                    # epilogue chains run in parallel instead of queuing on DVE
                    nc.scalar.copy(lsb[:], zps[Dh:Dh + 1, :])
                else:
                    nc.vector.tensor_copy(lsb[:], zps[Dh:Dh + 1, :])
                lsbb = sm_pool.tile([64, QB], F32, tag="lsbb")
                nc.gpsimd.partition_broadcast(lsbb[:], lsb[:], channels=64)
                rsb = sm_pool.tile([64, QB], F32, tag="rsb")
                nc.vector.reciprocal_approx_fast(rsb[:], lsbb[:])
                nc.vector.tensor_mul(znt[p0:p0 + 64, ce * S + q0:ce * S + q0 + QB],
                                     zps[0:64, :], rsb[:])

            # ---- main j-major loop: head PAIRS, concurrent half-array scores ----
            for j in range(NQB):
                fillers = []
                if j + 1 < NQB:
                    fillers = rr(qk_proj_units(j + 1),
                                 v_proj_units(4 * (j + 1)), v_proj_units(4 * (j + 1) + 1),
                                 v_proj_units(4 * (j + 1) + 2), v_proj_units(4 * (j + 1) + 3))
                wo_fill = []
                if j == NQB - 1:
                    wo_fill = [u for st2 in range(4 * (NQB - 1)) for u in wo_units(st2)]
                fq = list(fillers)
                wq_ = list(wo_fill)

                q0 = j * QB
                nkt = 4 * (j + 1)
                total_k = NC_E * nkt
                k_idx = 0
                for pr in range(NC_E):
                    ce = pr
                    hA, hB = 2 * pr, 2 * pr + 1
                    zpsA = ps_z.tile([128, QB], F32, tag="z", name="zpsA")
                    zpsB = ps_z.tile([128, QB], F32, tag="z", name="zpsB")
                    for k in range(nkt):
                        qoff = 128 * (k - 4 * j) if k >= 4 * j else 0
                        ns = QB - qoff
                        # both heads' scores side by side in ONE psum tile so a
                        # single exp frees the slot: the two 64-row score MMs
                        # have identical deps and dispatch concurrently into
                        # different PE row-groups
                        # B head at fixed offset QB so its output stays within
                        # one PSUM bank even for trimmed diagonal tiles
                        pss = ps_s.tile([128, 2 * QB], F32, tag="s", name="pss")
                        if j >= 2:
                            # anti-throttle: dead matmul with no data deps keeps
                            # the HAM clock gate warm through the exp-wait gap;
                            # the real score MM below overwrites it (start=True)
                            nc.tensor.matmul(pss[:, 0:64], wscr[:, 0:128],
                                             wscr[:, 0:64], start=True, stop=True,
                                             skip_group_check=True)
                        lhA = kt[0:64, ce * S + k * 128:ce * S + k * 128 + 128]
                        lhB = kt[64:128, ce * S + k * 128:ce * S + k * 128 + 128]
                        rhA = qt[0:64, ce * S + q0 + qoff:ce * S + q0 + QB]
                        rhB = qt[64:128, ce * S + q0 + qoff:ce * S + q0 + QB]
                        nc.tensor.matmul(pss[:, 0:ns], lhA, rhA,
                                         start=True, stop=True, skip_group_check=True)
                        nc.tensor.matmul(pss[:, QB:QB + ns], lhB, rhB,
                                         start=True, stop=True, skip_group_check=True)
                        pt = pt_pool.tile([128, 2 * QB], BF16, name="pt")
                        # one activation covering [0, QB+ns): the [ns, QB) gap
                        # holds stale-score garbage whose exp lands in pt cols
                        # no PV matmul ever reads (saves a 293ns instr per
                        # diagonal tile)
                        nc.scalar.activation(pt[:, 0:QB + ns], pss[:, 0:QB + ns],
                                             Exp, scale=SCALE8)
                        if k >= 4 * j:
                            nc.vector.tensor_mul(pt[:, 0:128], pt[:, 0:128], mask[:])
                            nc.vector.tensor_mul(pt[:, QB:QB + 128],
                                                 pt[:, QB:QB + 128], mask[:])
                        vaA = va[:, k * HL * VW + hA * VW:k * HL * VW + hA * VW + VW]
                        vaB = va[:, k * HL * VW + hB * VW:k * HL * VW + hB * VW + VW]
                        nc.tensor.matmul(zpsA[0:VW, qoff:QB], vaA, pt[:, 0:ns],
                                         start=(k == 0), stop=(k == nkt - 1),
                                         skip_group_check=True)
                        nc.tensor.matmul(zpsB[0:VW, qoff:QB], vaB, pt[:, QB:QB + ns],
                                         start=(k == 0), stop=(k == nkt - 1),
                                         skip_group_check=True)
                        # interleave filler matmuls to keep PE dense
                        k_idx += 1
                        rem = total_k - k_idx
                        avail = len(fq) + len(wq_)
                        take = -(-avail // max(rem, 1)) if avail else 0
                        for _ in range(take):
                            if fq:
                                fq.pop(0)()
                            elif wq_:
                                wq_.pop(0)()
                    last = (j == NQB - 1) and (pr == NC_E - 1)
                    epilogue(hA, j, zpsA)
                    epilogue(hB, j, zpsB, sc_copy=last)
                    # a couple of fillers right at the pair boundary bridge the
                    # epilogue-chain latency before the next pair's PV can start
                    for _ in range(2):
                        if fq:
                            fq.pop(0)()
                        elif wq_:
                            wq_.pop(0)()
                # flush any leftover fillers for this j
                for u in fq:
                    u()
                for u in wq_:
                    u()
            for st2 in range(4 * (NQB - 1), NKT):
                for u in wo_units(st2):
                    u()

    nc.compile()
    return nc


def _in_maps(inputs):
    residual = np.asarray(inputs["residual"], np.float32)
    W_Q = np.asarray(inputs["W_Q"], np.float32)
    W_K = np.asarray(inputs["W_K"], np.float32)
    W_V = np.asarray(inputs["W_V"], np.float32)
    W_O = np.asarray(inputs["W_O"], np.float32)
    b_Q = np.asarray(inputs["b_Q"], np.float32)
    b_K = np.asarray(inputs["b_K"], np.float32)
    b_V = np.asarray(inputs["b_V"], np.float32)
    mask = (np.arange(128)[:, None] <= np.arange(128)[None, :]).astype(ml_dtypes.bfloat16)
    maps = []
    for c in range(8):
        b, g = c // 2, c % 2
        hs = slice(HL * g, HL * g + HL)
        xtf = np.ascontiguousarray(residual[b].T)
        xt = xtf.astype(ml_dtypes.bfloat16)
        xt8 = np.clip(xtf, -240, 240).astype(ml_dtypes.float8_e4m3fn)
        wqf = np.ascontiguousarray(np.transpose(W_Q[hs], (1, 0, 2)).reshape(D, NE))
        wkf = np.ascontiguousarray(np.transpose(W_K[hs], (1, 0, 2)).reshape(D, NE))
        wqm = np.clip(wqf * W8SC, -240, 240).astype(ml_dtypes.float8_e4m3fn)
        wkm = np.clip(wkf * W8SC, -240, 240).astype(ml_dtypes.float8_e4m3fn)
        wvm = np.ascontiguousarray(np.transpose(W_V[hs], (1, 0, 2)).reshape(D, NE)).astype(ml_dtypes.bfloat16)
        wom = np.ascontiguousarray(W_O[hs].reshape(NE, D)).astype(ml_dtypes.bfloat16)
        bqm = np.ascontiguousarray(b_Q[hs].reshape(NC_E, 128).T) * W8SC
        bkm = np.ascontiguousarray(b_K[hs].reshape(NC_E, 128).T) * W8SC
        bvm = np.zeros((128, HL * VW), np.float32)
        for h in range(HL):
            bvm[:, h * VW:h * VW + Dh] = b_V[HL * g + h][None, :]
        maps.append({"xt": xt, "xt8": xt8, "wq": wqm, "wk": wkm, "wv": wvm,
                     "wo": wom, "bq": bqm, "bk": bkm, "bv": bvm, "mask": mask})
    return maps


def _run(inputs, trace=False, **kw):
    if "nc" not in _CACHE:
        _CACHE["nc"] = _build()
    nc = _CACHE["nc"]
    res = bass_utils.run_bass_kernel_spmd(nc, _in_maps(inputs),
                                          core_ids=list(range(8)), trace=trace, **kw)
    b_O = np.asarray(inputs["b_O"], np.float32)
    out = np.empty((B, S, D), np.float32)
    for b in range(B):
        out[b] = res.results[2 * b]["out"] + res.results[2 * b + 1]["out"] + b_O
    return out, res


def kernel(**inputs):
    out, _ = _run(inputs)
    return out


# revision 48
# speedup vs baseline: 1.0491x; 1.0491x over previous
"""Causal multi-head attention (B=4, S=2048, D=768, H=12, Dh=64) on 8 TRN2 NeuronCores.

Sharding: B x head-group. Core c handles batch b = c//2, heads 6g..6g+5 with
g = c%2. Each core computes QKV projections for its 6 heads, causal
flash-style attention in scores-transposed layout, and a partial W_O
contraction. Host sums the two per-batch partials and adds b_O.

Structure: j-major over q-blocks, processing heads in PAIRS (even head in
partitions 0-63, odd head in 64-127) so the two score matmuls (contraction
Dh=64) run concurrently in different PE row-groups. Projection / W_O matmuls
interleave as fillers to keep TensorE dense. Warmup matmuls at t=0 warm the
HAM clock gate while input DMAs (split across both queues, d-chunk-major)
are in flight.

No collectives: per-core outputs are disjoint-summable partials.
"""
import sys

if "/opt/trn_rl_repo" not in sys.path:
    sys.path.insert(0, "/opt/trn_rl_repo")

import contextlib

import ml_dtypes
import numpy as np

import concourse.bass as bass
import concourse.tile as tile
from concourse import bacc, mybir
from concourse import bass_utils

F32 = mybir.dt.float32
F32R = mybir.dt.float32r
BF16 = mybir.dt.bfloat16
FP8 = mybir.dt.float8e4
DR = mybir.MatmulPerfMode.DoubleRow
Exp = mybir.ActivationFunctionType.Exp

B, S, D, H, Dh = 4, 2048, 768, 12, 64
HL = 6          # heads per core
NE = HL * Dh    # 384 he-dims per core
NC_D = D // 128   # 6 d chunks
NC_E = NE // 128  # 3 he chunks
QB = 512        # q block
NQB = S // QB   # 4
NKT = S // 128  # 16 k tiles
VW = Dh + 1     # 65: v + ones column
SCALE = 1.0 / np.sqrt(Dh)
W8SC = 32.0     # fp8 weight prescale; q,k come out 32x large
SCALE8 = SCALE / (W8SC * W8SC)  # folds the 32^2 back out inside the exp
NWARM = 10      # HAM warmup matmuls

_CACHE = {}


def _build():
    nc = bacc.Bacc("TRN2", target_bir_lowering=False, debug=False, num_devices=8)
    xt_d = nc.dram_tensor("xt", [D, S], BF16, kind="ExternalInput")
    xt8_d = nc.dram_tensor("xt8", [D, S], FP8, kind="ExternalInput")
    wq_d = nc.dram_tensor("wq", [D, NE], FP8, kind="ExternalInput")
    wk_d = nc.dram_tensor("wk", [D, NE], FP8, kind="ExternalInput")
    wv_d = nc.dram_tensor("wv", [D, NE], BF16, kind="ExternalInput")
    wo_d = nc.dram_tensor("wo", [NE, D], BF16, kind="ExternalInput")
    bq_d = nc.dram_tensor("bq", [128, NC_E], F32, kind="ExternalInput")
    bk_d = nc.dram_tensor("bk", [128, NC_E], F32, kind="ExternalInput")
    bv_d = nc.dram_tensor("bv", [128, HL * VW], F32, kind="ExternalInput")
    mask_d = nc.dram_tensor("mask", [128, 128], BF16, kind="ExternalInput")
    out_d = nc.dram_tensor("out", [S, D], F32, kind="ExternalOutput")

    with tile.TileContext(nc) as tc:
        with contextlib.ExitStack() as ctx:
            sb = ctx.enter_context(tc.tile_pool(name="sb", bufs=1))
            pt_pool = ctx.enter_context(tc.tile_pool(name="pt", bufs=6))
            sm_pool = ctx.enter_context(tc.tile_pool(name="sm", bufs=4))
            o_pool = ctx.enter_context(tc.tile_pool(name="o", bufs=3))
            ps_s = ctx.enter_context(tc.tile_pool(name="pss", bufs=2, space="PSUM"))
            ps_mm = ctx.enter_context(tc.tile_pool(name="psmm", bufs=2, space="PSUM"))
            ps_z = ctx.enter_context(tc.tile_pool(name="psz", bufs=2, space="PSUM"))

            # ---- persistent SBUF ----
            xt = sb.tile([128, NC_D * S], BF16, tag="xt")
            xt8 = sb.tile([128, NC_D * S], FP8, tag="xt8")
            wq = sb.tile([128, NC_D * NE], FP8, tag="wq")
            wk = sb.tile([128, NC_D * NE], FP8, tag="wk")
            wv = sb.tile([128, NC_D * NE], BF16, tag="wv")
            wo = sb.tile([128, NC_E * D], BF16, tag="wo")
            bq = sb.tile([128, NC_E], F32, tag="bq")
            bk = sb.tile([128, NC_E], F32, tag="bk")
            bvb = sb.tile([128, HL * VW], F32, tag="bvb")
            mask = sb.tile([128, 128], BF16, tag="mask")
            ones_b = sb.tile([1, 64], BF16, tag="ones_b")
            wscr = sb.tile([128, 512], BF16, tag="wscr")
            qt = sb.tile([128, NC_E * S], BF16, tag="qt")
            kt = sb.tile([128, NC_E * S], BF16, tag="kt")
            va = sb.tile([128, NKT * HL * VW], BF16, tag="va")
            znt = sb.tile([128, NC_E * S], BF16, tag="znt")

            # ---- input DMAs: d-chunk-major groups split across both queues ----
            wq_r = wq_d.ap().rearrange("(c p) e -> p c e", p=128)
            wk_r = wk_d.ap().rearrange("(c p) e -> p c e", p=128)
            wv_r = wv_d.ap().rearrange("(c p) e -> p c e", p=128)
            wq_s = wq[:].rearrange("p (c e) -> p c e", c=NC_D)
            wk_s = wk[:].rearrange("p (c e) -> p c e", c=NC_D)
            wv_s = wv[:].rearrange("p (c e) -> p c e", c=NC_D)
            xt_r = xt_d.ap().rearrange("(c p) s -> p c s", p=128)
            xt_s = xt[:].rearrange("p (c s) -> p c s", c=NC_D)
            xt8_r = xt8_d.ap().rearrange("(c p) s -> p c s", p=128)
            xt8_s = xt8[:].rearrange("p (c s) -> p c s", c=NC_D)

            def load_group(q, c):
                q.dma_start(wq_s[:, c:c + 1, :], wq_r[:, c:c + 1, :])
                q.dma_start(wk_s[:, c:c + 1, :], wk_r[:, c:c + 1, :])
                q.dma_start(wv_s[:, c:c + 1, :], wv_r[:, c:c + 1, :])
                q.dma_start(xt_s[:, c, 0:QB], xt_r[:, c, 0:QB])
                q.dma_start(xt8_s[:, c, 0:QB], xt8_r[:, c, 0:QB])

            # alternate d-chunk groups across the two queues so each DoubleRow
            # chunk-PAIR (2k, 2k+1) completes as early as possible
            load_group(nc.sync, 0)
            load_group(nc.gpsimd, 1)
            load_group(nc.sync, 2)
            load_group(nc.gpsimd, 3)
            nc.sync.dma_start(bq[:], bq_d.ap())
            nc.sync.dma_start(bk[:], bk_d.ap())
            load_group(nc.sync, 4)
            load_group(nc.gpsimd, 5)
            nc.sync.dma_start(mask[:], mask_d.ap())
            nc.gpsimd.dma_start(bvb[:], bv_d.ap())
            wo_r = wo_d.ap().rearrange("(c p) d -> p c d", p=128)
            wo_s = wo[:].rearrange("p (c d) -> p c d", c=NC_E)
            # remaining x quarters (needed from j0 fillers on), then W_O weights
            # (not needed until late j2)
            for qq in range(1, 4):
                s0 = qq * QB
                nc.sync.dma_start(xt_s[:, 0:3, s0:s0 + QB], xt_r[:, 0:3, s0:s0 + QB])
                nc.gpsimd.dma_start(xt_s[:, 3:6, s0:s0 + QB], xt_r[:, 3:6, s0:s0 + QB])
                nc.sync.dma_start(xt8_s[:, 0:3, s0:s0 + QB], xt8_r[:, 0:3, s0:s0 + QB])
                nc.gpsimd.dma_start(xt8_s[:, 3:6, s0:s0 + QB], xt8_r[:, 3:6, s0:s0 + QB])
            nc.gpsimd.dma_start(wo_s[:], wo_r[:])

            # ---- constants + HAM warmup (no DMA deps) ----
            nc.vector.memset(ones_b[:], 1.0)
            nc.vector.memset(wscr[:], 0.0)
            va_4d = va[:].rearrange("p (s h e) -> p s h e", s=NKT, h=HL)
            nc.vector.memset(va_4d[:, :, :, Dh:Dh + 1], 1.0)
            wps = ps_z.tile([128, QB], F32, tag="z", name="wps")
            for _ in range(NWARM):
                nc.tensor.matmul(wps[:], wscr[:, 0:128], wscr[:], start=True,
                                 stop=True, skip_group_check=True)

            # ---- prologue: QK+V projections for q-block 0 / k-tiles 0-3 ----
            # 3 QK chains (ce 0-2) + V chains, all kc-progressive so they
            # consume the d-chunk DMA groups as they land.
            psq0 = ps_mm.tile([128, QB], F32, tag="mm", name="psq0")
            psk0 = ps_mm.tile([128, QB], F32, tag="mm", name="psk0")
            qk1 = ps_s.tile([128, 2 * QB], F32, tag="s", name="qk1")
            qk2 = ps_s.tile([128, 2 * QB], F32, tag="s", name="qk2")
            pro_qk = {0: (psq0[:], psk0[:]),
                      1: (qk1[0:128, 0:QB], qk1[0:128, QB:2 * QB]),
                      2: (qk2[0:128, 0:QB], qk2[0:128, QB:2 * QB])}
            psvs = [ps_z.tile([128, NE], F32, tag="z", name=f"psv{st}")
                    for st in range(4)]

            def v_bias_epilogue(st, psv):
                o = st * HL * VW
                va_v = va[:, o:o + HL * VW].rearrange("p (h e) -> p h e", h=HL)[:, :, 0:Dh]
                ps_v = psv.rearrange("p (h e) -> p h e", h=HL)
                bv_v = bvb[:].rearrange("p (h e) -> p h e", h=HL)[:, :, 0:Dh]
                nc.vector.tensor_add(va_v, ps_v, bv_v)

            for kc in range(NC_D):
                st_, sp = (kc == 0), (kc == NC_D - 1)
                if kc % 2 == 1:
                    # fp8 DoubleRow: d-chunk pair (kc-1, kc) contracts in one MM
                    kcp = kc // 2
                    for ce in range(NC_E):
                        pq, pk = pro_qk[ce]
                        lq = wq_s[:, 2 * kcp:2 * kcp + 2, ce * 128:ce * 128 + 128]
                        lk = wk_s[:, 2 * kcp:2 * kcp + 2, ce * 128:ce * 128 + 128]
                        r8 = xt8_s[:, 2 * kcp:2 * kcp + 2, 0:QB]
                        nc.tensor.matmul(pq, lq, r8, start=(kcp == 0),
                                         stop=(kcp == NC_D // 2 - 1),
                                         perf_mode=DR, skip_group_check=True)
                        nc.tensor.matmul(pk, lk, r8, start=(kcp == 0),
                                         stop=(kcp == NC_D // 2 - 1),
                                         perf_mode=DR, skip_group_check=True)
                        if sp:
                            nc.vector.tensor_scalar_add(qt[:, ce * S:ce * S + QB],
                                                        pq, bq[:, ce:ce + 1])
                            nc.vector.tensor_scalar_add(kt[:, ce * S:ce * S + QB],
                                                        pk, bk[:, ce:ce + 1])
                for st in range(4):
                    lx = xt[:, kc * S + st * 128:kc * S + st * 128 + 128]
                    nc.tensor.matmul(psvs[st][:], lx, wv[:, kc * NE:kc * NE + NE],
                                     start=st_, stop=sp)
                    if sp:
                        v_bias_epilogue(st, psvs[st][:])

            # ---- filler work units (each emits one PE matmul + epilogue) ----
            def qk_proj_units(sblk):
                s0 = sblk * QB
                for ce in range(NC_E):
                    state = {}

                    def unit(ce=ce, state=state):
                        kcp = state.setdefault("kcp", 0)
                        if kcp == 0:
                            state["psq"] = ps_mm.tile([128, QB], F32, tag="mm", name="psq")
                            state["psk"] = ps_mm.tile([128, QB], F32, tag="mm", name="psk")
                        lq = wq_s[:, 2 * kcp:2 * kcp + 2, ce * 128:ce * 128 + 128]
                        lk = wk_s[:, 2 * kcp:2 * kcp + 2, ce * 128:ce * 128 + 128]
                        r8 = xt8_s[:, 2 * kcp:2 * kcp + 2, s0:s0 + QB]
                        st_, sp = (kcp == 0), (kcp == NC_D // 2 - 1)
                        nc.tensor.matmul(state["psq"][:], lq, r8, start=st_, stop=sp,
                                         perf_mode=DR)
                        nc.tensor.matmul(state["psk"][:], lk, r8, start=st_, stop=sp,
                                         perf_mode=DR)
                        if sp:
                            nc.vector.tensor_scalar_add(
                                qt[:, ce * S + s0:ce * S + s0 + QB], state["psq"][:],
                                bq[:, ce:ce + 1])
                            nc.vector.tensor_scalar_add(
                                kt[:, ce * S + s0:ce * S + s0 + QB], state["psk"][:],
                                bk[:, ce:ce + 1])
                        state["kcp"] = kcp + 1

                    for _ in range(NC_D // 2):
                        yield unit

            def v_proj_units(st):
                state = {}

                def unit(state=state):
                    kc = state.setdefault("kc", 0)
                    if kc == 0:
                        state["psv"] = ps_mm.tile([128, NE], F32, tag="mm", name="psv")
                    lx = xt[:, kc * S + st * 128:kc * S + st * 128 + 128]
                    nc.tensor.matmul(state["psv"][:], lx, wv[:, kc * NE:kc * NE + NE],
                                     start=(kc == 0), stop=(kc == NC_D - 1))
                    if kc == NC_D - 1:
                        v_bias_epilogue(st, state["psv"][:])
                    state["kc"] = kc + 1

                for _ in range(NC_D):
                    yield unit

            def wo_units(st):
                osb = [None]

                def mkunit(dh, c, osb=osb):
                    def unit():
                        if dh == 0 and c == 0:
                            osb[0] = o_pool.tile([128, D], F32, tag="osb", name="osb")
                        if c == 0:
                            osb.append(ps_mm.tile([128, 384], F32, tag="mm", name="pso"))
                        pso = osb[-1]
                        lhsT = znt[:, c * S + st * 128:c * S + st * 128 + 128]
                        rhs = wo[:, c * D + dh * 384:c * D + dh * 384 + 384]
                        nc.tensor.matmul(pso[:], lhsT, rhs, start=(c == 0),
                                         stop=(c == NC_E - 1))
                        if c == NC_E - 1:
                            nc.vector.tensor_copy(osb[0][:, dh * 384:dh * 384 + 384], pso[:])
                            nc.sync.dma_start(
                                out_d.ap()[st * 128:st * 128 + 128,
                                           dh * 384:dh * 384 + 384],
                                osb[0][:, dh * 384:dh * 384 + 384])
                    return unit

                for dh in range(2):
                    for c in range(NC_E):
                        yield mkunit(dh, c)

            def rr(*gens):
                gens = [iter(g) for g in gens]
                out = []
                while gens:
                    nxt = []
                    for g in gens:
                        try:
                            out.append(next(g))
                            nxt.append(g)
                        except StopIteration:
                            pass
                    gens = nxt
                return out

            # ---- epilogue: normalize z^T by the softmax denominators ----
            def epilogue(h, j, zps, sc_copy=False):
                ce, sub = h // 2, h % 2
                p0 = 64 * sub
                q0 = j * QB
                lsb = sm_pool.tile([1, QB], F32, tag="lsb")
                if sc_copy:
                    # ScalarE is idle at the very end; lets the two final
                    # epilogue chains run in parallel instead of queuing on DVE
                    nc.scalar.copy(lsb[:], zps[Dh:Dh + 1, :])
                else:
                    nc.vector.tensor_copy(lsb[:], zps[Dh:Dh + 1, :])
                lsbb = sm_pool.tile([64, QB], F32, tag="lsbb")
                nc.gpsimd.partition_broadcast(lsbb[:], lsb[:], channels=64)
                rsb = sm_pool.tile([64, QB], F32, tag="rsb")
                nc.vector.reciprocal_approx_fast(rsb[:], lsbb[:])
                nc.vector.tensor_mul(znt[p0:p0 + 64, ce * S + q0:ce * S + q0 + QB],
                                     zps[0:64, :], rsb[:])

            # ---- main j-major loop: head PAIRS, concurrent half-array scores ----
            for j in range(NQB):
                fillers = []
                if j + 1 < NQB:
                    fillers = rr(qk_proj_units(j + 1),
                                 v_proj_units(4 * (j + 1)), v_proj_units(4 * (j + 1) + 1),
                                 v_proj_units(4 * (j + 1) + 2), v_proj_units(4 * (j + 1) + 3))
                wo_fill = []
                if j == NQB - 1:
                    wo_fill = [u for st2 in range(4 * (NQB - 1)) for u in wo_units(st2)]
                fq = list(fillers)
                wq_ = list(wo_fill)

                q0 = j * QB
                nkt = 4 * (j + 1)
                total_k = NC_E * nkt
                k_idx = 0
                for pr in range(NC_E):
                    ce = pr
                    hA, hB = 2 * pr, 2 * pr + 1
                    zpsA = ps_z.tile([128, QB], F32, tag="z", name="zpsA")
                    zpsB = ps_z.tile([128, QB], F32, tag="z", name="zpsB")
                    for k in range(nkt):
                        qoff = 128 * (k - 4 * j) if k >= 4 * j else 0
                        ns = QB - qoff
                        # both heads' scores side by side in ONE psum tile so a
                        # single exp frees the slot: the two 64-row score MMs
                        # have identical deps and dispatch concurrently into
                        # different PE row-groups
                        # B head at fixed offset QB so its output stays within
                        # one PSUM bank even for trimmed diagonal tiles
                        pss = ps_s.tile([128, 2 * QB], F32, tag="s", name="pss")
                        if j >= 2:
                            # anti-throttle: dead matmul with no data deps keeps
                            # the HAM clock gate warm through the exp-wait gap;
                            # the real score MM below overwrites it (start=True)
                            nc.tensor.matmul(pss[:, 0:64], wscr[:, 0:128],
                                             wscr[:, 0:64], start=True, stop=True,
                                             skip_group_check=True)
                        lhA = kt[0:64, ce * S + k * 128:ce * S + k * 128 + 128]
                        lhB = kt[64:128, ce * S + k * 128:ce * S + k * 128 + 128]
                        rhA = qt[0:64, ce * S + q0 + qoff:ce * S + q0 + QB]
                        rhB = qt[64:128, ce * S + q0 + qoff:ce * S + q0 + QB]
                        nc.tensor.matmul(pss[:, 0:ns], lhA, rhA,
                                         start=True, stop=True, skip_group_check=True)
                        nc.tensor.matmul(pss[:, QB:QB + ns], lhB, rhB,
                                         start=True, stop=True, skip_group_check=True)
                        pt = pt_pool.tile([128, 2 * QB], BF16, name="pt")
                        # one activation covering [0, QB+ns): the [ns, QB) gap
                        # holds stale-score garbage whose exp lands in pt cols
                        # no PV matmul ever reads (saves a 293ns instr per
                        # diagonal tile)
                        nc.scalar.activation(pt[:, 0:QB + ns], pss[:, 0:QB + ns],
                                             Exp, scale=SCALE8)
                        if k >= 4 * j:
                            nc.vector.tensor_mul(pt[:, 0:128], pt[:, 0:128], mask[:])
                            nc.vector.tensor_mul(pt[:, QB:QB + 128],
                                                 pt[:, QB:QB + 128], mask[:])
                        vaA = va[:, k * HL * VW + hA * VW:k * HL * VW + hA * VW + VW]
                        vaB = va[:, k * HL * VW + hB * VW:k * HL * VW + hB * VW + VW]
                        nc.tensor.matmul(zpsA[0:VW, qoff:QB], vaA, pt[:, 0:ns],
                                         start=(k == 0), stop=(k == nkt - 1),
                                         skip_group_check=True)
                        nc.tensor.matmul(zpsB[0:VW, qoff:QB], vaB, pt[:, QB:QB + ns],
                                         start=(k == 0), stop=(k == nkt - 1),
                                         skip_group_check=True)
                        # interleave filler matmuls to keep PE dense
                        k_idx += 1
                        rem = total_k - k_idx
                        avail = len(fq) + len(wq_)
                        take = -(-avail // max(rem, 1)) if avail else 0
                        for _ in range(take):
                            if fq:
                                fq.pop(0)()
                            elif wq_:
                                wq_.pop(0)()
                    last = (j == NQB - 1) and (pr == NC_E - 1)
                    epilogue(hA, j, zpsA)
                    epilogue(hB, j, zpsB, sc_copy=last)
                    # a couple of fillers right at the pair boundary bridge the
                    # epilogue-chain latency before the next pair's PV can start
                    for _ in range(2):
                        if fq:
                            fq.pop(0)()
                        elif wq_:
                            wq_.pop(0)()
                # flush any leftover fillers for this j
                for u in fq:
                    u()
                for u in wq_:
                    u()
            for st2 in range(4 * (NQB - 1), NKT):
                for u in wo_units(st2):
                    u()

    nc.compile()
    return nc


def _in_maps(inputs):
    residual = np.asarray(inputs["residual"], np.float32)
    W_Q = np.asarray(inputs["W_Q"], np.float32)
    W_K = np.asarray(inputs["W_K"], np.float32)
    W_V = np.asarray(inputs["W_V"], np.float32)
    W_O = np.asarray(inputs["W_O"], np.float32)
    b_Q = np.asarray(inputs["b_Q"], np.float32)
    b_K = np.asarray(inputs["b_K"], np.float32)
    b_V = np.asarray(inputs["b_V"], np.float32)
    mask = (np.arange(128)[:, None] <= np.arange(128)[None, :]).astype(ml_dtypes.bfloat16)
    maps = []
    for c in range(8):
        b, g = c // 2, c % 2
        hs = slice(HL * g, HL * g + HL)
        xtf = np.ascontiguousarray(residual[b].T)
        xt = xtf.astype(ml_dtypes.bfloat16)
        xt8 = np.clip(xtf, -240, 240).astype(ml_dtypes.float8_e4m3fn)
        wqf = np.ascontiguousarray(np.transpose(W_Q[hs], (1, 0, 2)).reshape(D, NE))
        wkf = np.ascontiguousarray(np.transpose(W_K[hs], (1, 0, 2)).reshape(D, NE))
        wqm = np.clip(wqf * W8SC, -240, 240).astype(ml_dtypes.float8_e4m3fn)
        wkm = np.clip(wkf * W8SC, -240, 240).astype(ml_dtypes.float8_e4m3fn)
        wvm = np.ascontiguousarray(np.transpose(W_V[hs], (1, 0, 2)).reshape(D, NE)).astype(ml_dtypes.bfloat16)
        wom = np.ascontiguousarray(W_O[hs].reshape(NE, D)).astype(ml_dtypes.bfloat16)
        bqm = np.ascontiguousarray(b_Q[hs].reshape(NC_E, 128).T) * W8SC
        bkm = np.ascontiguousarray(b_K[hs].reshape(NC_E, 128).T) * W8SC
        bvm = np.zeros((128, HL * VW), np.float32)
        for h in range(HL):
            bvm[:, h * VW:h * VW + Dh] = b_V[HL * g + h][None, :]
        maps.append({"xt": xt, "xt8": xt8, "wq": wqm, "wk": wkm, "wv": wvm,
                     "wo": wom, "bq": bqm, "bk": bkm, "bv": bvm, "mask": mask})
    return maps


def _run(inputs, trace=False, **kw):
    if "nc" not in _CACHE:
        _CACHE["nc"] = _build()
    nc = _CACHE["nc"]
    res = bass_utils.run_bass_kernel_spmd(nc, _in_maps(inputs),
                                          core_ids=list(range(8)), trace=trace, **kw)
    b_O = np.asarray(inputs["b_O"], np.float32)
    out = np.empty((B, S, D), np.float32)
    for b in range(B):
        out[b] = res.results[2 * b]["out"] + res.results[2 * b + 1]["out"] + b_O
    return out, res


def kernel(**inputs):
    out, _ = _run(inputs)
    return out
